# revision 1
# baseline (speedup 1.0000x reference)
"""Trainium2 Bass kernel for nn_Block_69191923139027 (dense_transformer).

Sharding: 8 cores; core k handles Feebler/Booster rows i in [8k, 8k+8) for
all batches. AllGather collectives stitch the per-batch global reductions
(ck/cv, softmax denominator) and the full h_final needed by the Booster.

v3: merged 4MB DMAs (x/fw/bw/out via strided APs), per-batch pipelines on
both sides (feebler->LN1->qkv and proj->LN2->FFN->tok), softmax scale
folded into ACT-exp scale and proj weights, rstd = exp(-0.5*ln(var+eps)),
bf16 feebler-reduce matmuls, constant-shift softmax (max |logit| ~71 < 88).

Self-contained: hardcodes all shapes; no sibling imports.
"""

import numpy as np

import concourse.bacc as bacc
import concourse.mybir as mybir
import concourse.tile as tile
from concourse.bass_utils import run_bass_kernel_spmd

N_CORES = 8
B, T, SD, NE = 4, 2048, 64, 4096
H, HS, FH = 8, 8, 256
EPS = 1e-5
IPC = SD // N_CORES          # 8 feebler rows per core
TLOC = B * IPC * 32          # 1024 local tokens; hT col = b*256 + a*8 + i
DT = mybir.dt.float32
BF = mybir.dt.bfloat16
RG = [list(range(N_CORES))]
ESHIFT = 64.0                # softmax logit shift (max |logit| ~ 71)

_CACHE = {}


def _build_nc():
    nc = bacc.Bacc("TRN2", target_bir_lowering=False, debug=False,
                   num_devices=N_CORES)
    A = mybir.AluOpType
    AF = mybir.ActivationFunctionType

    tn = {}
    tn["x"] = nc.dram_tensor("x", [B * IPC * SD, T], DT, kind="ExternalInput")
    tn["fw"] = nc.dram_tensor("fw", [IPC * SD, T], DT, kind="ExternalInput")
    tn["bw"] = nc.dram_tensor("bw", [IPC * SD, T], DT, kind="ExternalInput")
    tn["wqkv"] = nc.dram_tensor("wqkv", [SD, 3 * SD], DT, kind="ExternalInput")
    tn["pw"] = nc.dram_tensor("pw", [SD, SD], DT, kind="ExternalInput")
    tn["pb"] = nc.dram_tensor("pb", [SD, 1], DT, kind="ExternalInput")
    tn["l1g"] = nc.dram_tensor("l1g", [SD, 1], DT, kind="ExternalInput")
    tn["l1b"] = nc.dram_tensor("l1b", [SD, 1], DT, kind="ExternalInput")
    tn["l2g"] = nc.dram_tensor("l2g", [SD, 1], DT, kind="ExternalInput")
    tn["l2b"] = nc.dram_tensor("l2b", [SD, 1], DT, kind="ExternalInput")
    tn["w1"] = nc.dram_tensor("w1", [SD, FH], DT, kind="ExternalInput")
    tn["b1h"] = nc.dram_tensor("b1h", [128, 2], DT, kind="ExternalInput")
    tn["w2"] = nc.dram_tensor("w2", [FH, SD], DT, kind="ExternalInput")
    tn["b2"] = nc.dram_tensor("b2", [SD, 1], DT, kind="ExternalInput")
    tn["eye64"] = nc.dram_tensor("eye64", [64, 64], DT, kind="ExternalInput")
    out = nc.dram_tensor("out", [B * IPC * SD, T], DT, kind="ExternalOutput")

    with tile.TileContext(nc) as tc:
        _body(nc, tc, tn, out, A, AF)
    nc.compile()
    return nc


def _body(nc, tc, tn, out, A, AF):
    X = mybir.AxisListType.X
    T4 = 4 * T  # 8192

    with tc.tile_pool(name="wconst", bufs=1) as wp, \
         tc.tile_pool(name="mid", bufs=1) as mp, \
         tc.tile_pool(name="bwpool", bufs=1) as bwp, \
         tc.tile_pool(name="dram", bufs=1, space="DRAM") as dp:

        # ---- on-chip constants (no DMA traffic) ----
        ones2 = wp.tile([128, 2], DT, tag="ones2")
        nc.vector.memset(ones2[:], 0.0)
        nc.vector.memset(ones2[0:64, 0:1], 1.0)
        nc.vector.memset(ones2[64:128, 1:2], 1.0)
        ones64 = wp.tile([SD, 1], DT, tag="ones64")
        nc.vector.memset(ones64[:], 1.0 / SD)
        epsv = wp.tile([64, 1], DT, tag="epsv")
        nc.vector.memset(epsv[:], EPS)
        neg64 = wp.tile([64, 1], DT, tag="neg64")
        nc.vector.memset(neg64[:], -ESHIFT)

        hTb = [mp.tile([64, 256], DT, tag=f"hT{b}", name=f"hT{b}")
               for b in range(B)]
        y1b = [mp.tile([64, 256], DT, tag=f"y1{b}", name=f"y1{b}")
               for b in range(B)]
        eTb = [mp.tile([64, 256], DT, tag=f"eT{b}", name=f"eT{b}")
               for b in range(B)]
        part = mp.tile([64, 4], DT, tag="part")   # ysum partials per batch

        wtiles = {}

        def wtile(name, shape, src):
            t = wp.tile(shape, DT, tag=name, name=f"w_{name}")
            nc.sync.dma_start(t[:], src)
            wtiles[name] = t
            return t

        def emit_small_weights():
            wtile("wqkv", [SD, 3 * SD], tn["wqkv"][:])
            wtile("pw", [SD, SD], tn["pw"][:])
            wtile("pb", [SD, 1], tn["pb"][:])
            wtile("l1g", [SD, 1], tn["l1g"][:])
            wtile("l1b", [SD, 1], tn["l1b"][:])
            wtile("l2g", [SD, 1], tn["l2g"][:])
            wtile("l2b", [SD, 1], tn["l2b"][:])
            wtile("w1", [SD, FH], tn["w1"][:])
            wtile("b1h", [128, 2], tn["b1h"][:])
            wtile("w2a", [128, SD], tn["w2"][0:128, :])
            wtile("w2b", [128, SD], tn["w2"][128:256, :])
            wtile("b2", [SD, 1], tn["b2"][:])
            wtile("eye64", [64, 64], tn["eye64"][:])

        # layer norm on [64, W] slice; aux tiles from given pools.
        # rstd = exp(-0.5*ln(var+eps)) keeps the hot path on ACT.
        def layer_norm(y_out, h_ap, g, bta, W, pls, lnp):
            sq = lnp.tile([64, W], DT, tag="ln_sq")
            nc.vector.tensor_mul(sq[:], h_ap, h_ap)
            stats_ps = pls.tile([1, 2 * W], DT, tag="ln_stats", bufs=2)
            mean_ps = stats_ps[:, 0:W]
            msq_ps = stats_ps[:, W:2 * W]
            for c in range(0, W, 512):
                sl = slice(c, min(c + 512, W))
                slm = slice(W + c, W + min(c + 512, W))
                nc.tensor.matmul(stats_ps[:, sl], ones64[:], h_ap[:, sl],
                                 start=True, stop=True)
                nc.tensor.matmul(stats_ps[:, slm], ones64[:], sq[:, sl],
                                 start=True, stop=True)
            mean_sb = lnp.tile([1, W], DT, tag="ln_mean_sb")
            nc.scalar.copy(mean_sb[:], mean_ps)
            mbsq = lnp.tile([1, W], DT, tag="ln_mbsq")
            nc.scalar.square(mbsq[:], mean_ps)
            var = lnp.tile([1, W], DT, tag="ln_var")
            nc.vector.tensor_sub(var[:], msq_ps, mbsq[:])
            lnv = lnp.tile([1, W], DT, tag="ln_lnv")
            nc.scalar.activation(lnv[:], var[:], AF.Ln, bias=epsv[0:1, 0:1])
            rstd = lnp.tile([1, W], DT, tag="ln_rstd")
            nc.scalar.activation(rstd[:], lnv[:], AF.Exp, scale=-0.5)
            meanb = lnp.tile([64, W], DT, tag="ln_meanb")
            nc.gpsimd.partition_broadcast(meanb[:], mean_sb[:])
            rstdb = lnp.tile([64, W], DT, tag="ln_rstdb")
            nc.gpsimd.partition_broadcast(rstdb[:], rstd[:])
            ymm = lnp.tile([64, W], DT, tag="ln_ymm")
            nc.vector.tensor_sub(ymm[:], h_ap, meanb[:])
            nc.vector.scalar_tensor_tensor(y_out, ymm[:], g[:, 0:1], rstdb[:],
                                           op0=A.mult, op1=A.mult)
            nc.vector.tensor_scalar_add(y_out, y_out, bta[:, 0:1])

        # ======== Phase A+B1: per-batch feebler -> transpose -> LN1 -> qkv
        with nc.named_scope("feebler"), \
             tc.tile_pool(name="fw", bufs=1) as fwp, \
             tc.tile_pool(name="xin", bufs=2) as xp, \
             tc.tile_pool(name="prod", bufs=2) as prp, \
             tc.tile_pool(name="ln1t", bufs=2) as lnp1, \
             tc.tile_pool(name="psAB", bufs=1, space="PSUM") as psAB:
            fwt = fwp.tile([128, T4], DT, tag="fwt")
            for b in range(B):
                xt = xp.tile([128, T4], DT, tag="x")
                if b == 0:
                    # interleave fw/x 1MB chunks so compute starts early
                    for m in range(4):
                        msl = slice(m * T, (m + 1) * T)
                        nc.sync.dma_start(
                            fwt[:, msl],
                            tn["fw"][2 * m * 64:(2 * m + 2) * 64, :])
                        nc.sync.dma_start(
                            xt[:, msl], tn["x"][m * 128:(m + 1) * 128, :])
                else:
                    nc.sync.dma_start(
                        xt[:], tn["x"][b * 512:(b + 1) * 512, :].rearrange(
                            "(m p) t -> p m t", m=4))
                dstv_all = hTb[b][:].rearrange("p (c e) -> p c e", c=16)
                for m in range(4):
                    msl = slice(m * T, (m + 1) * T)
                    prbf = prp.tile([128, T], DT, tag="prbf")
                    nc.vector.tensor_mul(prbf[:], xt[:, msl], fwt[:, msl])
                    # j-reduce with output already transposed: out rows are
                    # (a-parity, s), cols are the i-pair
                    hT2 = psAB.tile([128, 32], DT, tag="hT2", bufs=2)
                    for c in range(16):
                        nc.tensor.matmul(hT2[:, 2 * c:2 * c + 2],
                                         prbf[:, c * 128:(c + 1) * 128],
                                         ones2[:], start=True, stop=True)
                    for apar in range(2):
                        srcv = hT2[apar * 64:apar * 64 + 64, :].rearrange(
                            "p (c i) -> p c i", c=16)
                        e0 = 8 * apar + 2 * m
                        nc.scalar.copy(dstv_all[:, :, e0:e0 + 2], srcv)
                if b == 0:
                    emit_small_weights()
                if b == 3:
                    pass
                # -- LN1(b); k/v global sums come from AllGather(sum y)
                # via linearity: ck = wk^T @ sum(y), cv = wv^T @ sum(y)
                layer_norm(y1b[b][:], hTb[b][:], wtiles["l1g"],
                           wtiles["l1b"], 256, psAB, lnp1)
                nc.vector.tensor_reduce(part[:, b:b + 1], y1b[b][:],
                                        axis=X, op=A.add)

        # ======== AG1: y-sum partials -> global; q matmul overlaps it
        cc1i = dp.tile([64, 4], DT, tag="cc1i")
        cc1o = dp.tile([512, 4], DT, tag="cc1o", addr_space="Shared")
        nc.sync.dma_start(cc1i[:], part[:])
        nc.gpsimd.collective_compute("AllGather", A.bypass, ins=[cc1i[:]],
                                     outs=[cc1o[:]], replica_groups=RG)
        zp = mp.tile([64, 4], DT, tag="zp")
        with nc.named_scope("softmax"), \
             tc.tile_pool(name="psQ", bufs=1, space="PSUM") as psq:
            q_ps = psq.tile([64, TLOC], DT, tag="q_ps")
            for b in range(B):
                sl = slice(b * 256, (b + 1) * 256)
                nc.tensor.matmul(q_ps[:, sl], wtiles["wqkv"][:, 0:64],
                                 y1b[b][:], start=True, stop=True)
            gath = mp.tile([64, 32], DT, tag="gath")   # (batch 4, rank 8)
            nc.sync.dma_start(
                gath[:].rearrange("p (s r) -> p s r", s=4),
                cc1o[:].rearrange("(r p) s -> p s r", r=N_CORES))
            ysum = mp.tile([64, 4], DT, tag="ysum")
            nc.vector.tensor_reduce(ysum[:],
                                    gath[:].rearrange("p (s r) -> p s r",
                                                      s=4),
                                    axis=X, op=A.add)
            kv_ps = psq.tile([128, 4], DT, tag="kv_ps")
            nc.tensor.matmul(kv_ps[:], wtiles["wqkv"][:, 64:192], ysum[:],
                             start=True, stop=True)
            kvg = mp.tile([128, 4], DT, tag="kvg")   # ck rows 0:64, cv 64:128
            nc.scalar.copy(kvg[:], kv_ps[:])
            # e = exp(q*ck - 64); accumulate local softmax denominator
            for b in range(B):
                sl = slice(b * 256, (b + 1) * 256)
                nc.scalar.activation(eTb[b][:], q_ps[:, sl], AF.Exp,
                                     bias=neg64[:, 0:1],
                                     scale=kvg[0:64, b:b + 1],
                                     accum_out=zp[:, b:b + 1])
        # AG2: softmax denominator
        cc2i = dp.tile([64, 4], DT, tag="cc2i")
        cc2o = dp.tile([512, 4], DT, tag="cc2o", addr_space="Shared")
        nc.sync.dma_start(cc2i[:], zp[:])
        nc.gpsimd.collective_compute("AllGather", A.bypass, ins=[cc2i[:]],
                                     outs=[cc2o[:]], replica_groups=RG)
        gath2 = mp.tile([64, 32], DT, tag="gath2")
        nc.sync.dma_start(gath2[:].rearrange("p (s r) -> p s r", s=4),
                          cc2o[:].rearrange("(r p) s -> p s r", r=N_CORES))
        zg = mp.tile([64, 4], DT, tag="zg")
        nc.vector.tensor_reduce(zg[:],
                                gath2[:].rearrange("p (s r) -> p s r", s=4),
                                axis=X, op=A.add)
        rz = mp.tile([64, 4], DT, tag="rz")
        nc.vector.reciprocal(rz[:], zg[:])
        cvg = mp.tile([64, 4], DT, tag="cvg")
        nc.scalar.copy(cvg[:], kvg[64:128, :])
        sc = mp.tile([64, 4], DT, tag="sc")
        nc.vector.tensor_mul(sc[:], cvg[:], rz[:])

        # ======== per-batch: proj (sc folded into weights) -> LN2 -> FFN
        # ======== -> token-major transposes -> split AllGather
        cc3i = [dp.tile([8, 2048], DT, tag=f"cc3i{g}", name=f"cc3i{g}")
                for g in range(B)]
        cc3o = [dp.tile([8, 16384], DT, tag=f"cc3o{g}", name=f"cc3o{g}",
                        addr_space="Shared") for g in range(B)]
        with nc.named_scope("post"), \
             tc.tile_pool(name="postt", bufs=2) as pot, \
             tc.tile_pool(name="psPO", bufs=1, space="PSUM") as pps:
            eye64 = wtiles["eye64"]
            for b in range(B):
                pwb = pot.tile([64, 64], DT, tag="pwb")
                nc.vector.tensor_scalar_mul(pwb[:], wtiles["pw"][:],
                                            sc[:, b:b + 1])
                pj = pps.tile([64, 256], DT, tag="pj", bufs=2)
                nc.tensor.matmul(pj[:], pwb[:], eTb[b][:],
                                 start=True, stop=True)
                nc.vector.scalar_tensor_tensor(hTb[b][:], pj[:],
                                               wtiles["pb"][:, 0:1],
                                               hTb[b][:],
                                               op0=A.add, op1=A.add)
                y2 = pot.tile([64, 256], DT, tag="y2")
                layer_norm(y2[:], hTb[b][:], wtiles["l2g"], wtiles["l2b"],
                           256, pps, pot)
                f1a = pps.tile([128, 256], DT, tag="f1a")
                f1b = pps.tile([128, 256], DT, tag="f1b")
                nc.tensor.matmul(f1a[:], wtiles["w1"][:, 0:128], y2[:],
                                 start=True, stop=True)
                nc.tensor.matmul(f1b[:], wtiles["w1"][:, 128:256], y2[:],
                                 start=True, stop=True)
                r1a = pot.tile([128, 256], DT, tag="r1a")
                r1b = pot.tile([128, 256], DT, tag="r1b")
                nc.scalar.activation(r1a[:], f1a[:], AF.Relu,
                                     bias=wtiles["b1h"][:, 0:1])
                nc.scalar.activation(r1b[:], f1b[:], AF.Relu,
                                     bias=wtiles["b1h"][:, 1:2])
                f2 = pps.tile([64, 256], DT, tag="f2")
                nc.tensor.matmul(f2[:], wtiles["w2a"][:], r1a[:],
                                 start=True, stop=False)
                nc.tensor.matmul(f2[:], wtiles["w2b"][:], r1b[:],
                                 start=False, stop=True)
                nc.vector.scalar_tensor_tensor(hTb[b][:], f2[:],
                                               wtiles["b2"][:, 0:1],
                                               hTb[b][:],
                                               op0=A.add, op1=A.add)
                # token-major + stage into cc3i; per-batch AllGather
                for ah in range(2):
                    tp = pps.tile([128, 64], DT, tag="tok")
                    nc.tensor.transpose(
                        tp[:], hTb[b][:, ah * 128:(ah + 1) * 128], eye64[:])
                    tsb = pot.tile([128, 64], DT, tag="toksb")
                    nc.scalar.copy(tsb[:], tp[:])
                    dstv = cc3i[b][0:8,
                                   ah * 1024:ah * 1024 + 1024].rearrange(
                        "i (a s) -> a i s", a=16)
                    nc.sync.dma_start(dstv, tsb[:])
                nc.gpsimd.collective_compute(
                    "AllGather", A.bypass, ins=[cc3i[b][:]],
                    outs=[cc3o[b][:]], replica_groups=RG)

        # ======== Booster
        with nc.named_scope("booster"), \
             tc.tile_pool(name="hrb", bufs=1) as hrp, \
             tc.tile_pool(name="bprod", bufs=2) as bpp:
            bwt = bwp.tile([128, T4], DT, tag="bwt")
            nc.sync.dma_start(
                bwt[:], tn["bw"][:].rearrange("(m p) t -> p m t", m=4))
            hrbt = []
            for b in range(B):
                t = hrp.tile([128, T], DT, tag=f"hrb{b}", name=f"hrb{b}")
                src = cc3o[b][:].rearrange("r (j t) -> r j t", j=8)
                nc.sync.dma_start(t[0:64, :], src)
                nc.sync.dma_start(t[64:128, :], t[0:64, :])
                hrbt.append(t)
            for b in range(B):
                for m in range(4):
                    msl = slice(m * T, (m + 1) * T)
                    pr = bpp.tile([128, T], DT, tag="bprod", bufs=4)
                    if m == 0:
                        nc.gpsimd.tensor_mul(pr[:], bwt[:, msl], hrbt[b][:])
                    else:
                        nc.vector.tensor_mul(pr[:], bwt[:, msl], hrbt[b][:])
                    r0 = (b * 8 + 2 * m) * 64
                    nc.sync.dma_start(out[r0:r0 + 128, :], pr[:])


def _prep_host(inputs):
    """Host-side prep: shard x/fw/bw per core; pack small weights."""
    f32 = np.float32
    g = {k: np.asarray(v, dtype=f32) for k, v in inputs.items()}
    x = g["x"].reshape(B, SD, SD, T)          # flat view (b, i, j, t')
    fw, bw = g["feebler_w"], g["booster_w"]
    wq, wk, wv = g["wq"], g["wk"], g["wv"]
    wqkv = np.concatenate([w.transpose(1, 0, 2).reshape(SD, SD)
                           for w in (wq, wk, wv)], axis=1)  # [64, 192]
    shared = {
        "wqkv": np.ascontiguousarray(wqkv),
        "pw": np.ascontiguousarray(g["proj_w"]),
        "pb": g["proj_b"].reshape(SD, 1).copy(),
        "l1g": g["ln1_g"].reshape(SD, 1).copy(),
        "l1b": g["ln1_b"].reshape(SD, 1).copy(),
        "l2g": g["ln2_g"].reshape(SD, 1).copy(),
        "l2b": g["ln2_b"].reshape(SD, 1).copy(),
        "w1": np.ascontiguousarray(g["w1"]),
        "b1h": np.ascontiguousarray(g["b1"].reshape(2, 128).T),
        "w2": np.ascontiguousarray(g["w2"]),
        "b2": g["b2"].reshape(SD, 1).copy(),
        "eye64": np.eye(64, dtype=f32),
    }
    in_maps = []
    for k in range(N_CORES):
        i0 = k * IPC
        m = dict(shared)
        m["x"] = np.ascontiguousarray(
            x[:, i0:i0 + IPC].reshape(B * IPC * SD, T))
        m["fw"] = np.ascontiguousarray(
            fw[i0:i0 + IPC].reshape(IPC * SD, T))
        m["bw"] = np.ascontiguousarray(
            bw[i0:i0 + IPC].reshape(IPC * SD, T))
        in_maps.append(m)
    return in_maps


def _get_nc():
    if "nc" not in _CACHE:
        _CACHE["nc"] = _build_nc()
    return _CACHE["nc"]


def run(inputs, trace=False, **kw):
    nc = _get_nc()
    in_maps = _prep_host(inputs)
    res = run_bass_kernel_spmd(nc, in_maps, core_ids=list(range(N_CORES)),
                               trace=trace, **kw)
    full = np.empty((B, SD, SD, T), dtype=np.float32)
    for k in range(N_CORES):
        i0 = k * IPC
        full[:, i0:i0 + IPC] = res.results[k]["out"].reshape(B, IPC, SD, T)
    return full.reshape(B, T, NE), res


def kernel(**inputs):
    out, _ = run(inputs)
    return out

